# revision 1
# baseline (speedup 1.0000x reference)
"""ContrastiveLoss (margin=1) on 8 trn2 NeuronCores via Bass/Tile — v2.

Math: with d = cdist(output1, output2) [N, N], pos_r = rowmin(d),
pos_c = colmin(d), every hinge term  margin - pos + d >= margin > 0,
and the excluded (argmin) entry equals exactly margin.  Hence

  loss = (1 - 1/N) + sum(d)/N^2 - (mean(pos_r) + mean(pos_c))/2

Kernel needs sum(d), rowmin(d), colmin(d) in one pass over d.

Sharding: core c owns a 1024-row strip of output2 (b) and all of
output1 (a); computes e = dist(b_strip, a_full) [1024, 8192] with the
b index on partitions.  rowmin(e) (text-side pos_c) is local; colmin
needs a partition reduce (negate + gpsimd partition_all_reduce(max))
plus a cross-core ReduceScatter(max); the final scalar partials are
summed on the host from the 8 per-core outputs.

v2 changes vs baseline:
  - [128,1024] tiles (2 PSUM banks): half the per-op fixed costs,
    half the ACT accumulator reads (187ns each).
  - r1 = rowsum(a^2), r2 = rowsum(b^2) computed on HOST, shipped as
    inputs (kills 64 DVE squares + 64 ones-matmuls + 64 ACT copies).
  - ACT writes sqrt output directly into min-chain accumulator tiles
    (kills 16 init copies).
  - colmin partition-reduce via Pool partition_all_reduce (negate +
    max) instead of 64 PE transposes + 64 DVE reduces.
  - ReduceScatter(max on negated mins) instead of AllReduce(min):
    no 1.875x collective multiplier; host sums the 8 core scalars.
    (Pool/GPSIMD cannot run generic vector ops or touch PSUM on real
    HW -- only ISA ucode ops like partition_all_reduce.)
"""

import numpy as np
from contextlib import ExitStack

N = 8192          # rows of output1 == rows of output2
D = 128           # feature dim (== max matmul contraction)
NCORES = 8
R = N // NCORES   # 1024 rows per core
NIB = R // 128    # 8 row blocks per core (jb)
ST = 1024         # a-column strip width (one [128,1024] tile = 2 PSUM banks)
NST = N // ST     # 8 strips

MARGIN = 1.0
C0 = 1.0 / (float(N) * float(N))      # scale for sum(d)
C2 = -1.0 / (2.0 * float(N))          # scale for sum(pos_c)
C1N = 1.0 / (2.0 * float(N))          # scale for NEGATED sum(pos_r)
CONST = MARGIN - MARGIN / float(N)    # 1 - 1/8192  (added on host)

# engine assignment of the min chains (tuned against TimelineSim):
POOL_COLMIN_STRIPS = ()  # walrus: generic vector ops are NOT supported on Pool
POOL_ROWMIN_JBS = ()

_CACHE = {}


def _build():
    import concourse.bass as bass
    import concourse.bacc as bacc
    import concourse.tile as tile
    from concourse import mybir
    from concourse import bass_isa

    f32 = mybir.dt.float32
    f32r = mybir.dt.float32r
    bf16 = mybir.dt.bfloat16
    X = mybir.AxisListType.X
    MIN = mybir.AluOpType.min
    MAX = mybir.AluOpType.max
    ADD = mybir.AluOpType.add
    MULT = mybir.AluOpType.mult
    Sqrt = mybir.ActivationFunctionType.Sqrt

    nc = bacc.Bacc(
        trn_type="TRN2",
        target_bir_lowering=False,
        debug=False,
        num_devices=NCORES,
    )

    a_ext = nc.dram_tensor("a", [N, D], f32, kind="ExternalInput")
    b_ext = nc.dram_tensor("b", [R, D], f32, kind="ExternalInput")
    r1_ext = nc.dram_tensor("r1", [1, N], f32r, kind="ExternalInput")
    r2v_ext = nc.dram_tensor("r2v", [128, NIB], f32, kind="ExternalInput")
    out_ext = nc.dram_tensor("out", [1, 1], f32, kind="ExternalOutput")

    groups = [list(range(NCORES))]

    with tile.TileContext(nc) as tc, ExitStack() as ctx:
        const = ctx.enter_context(tc.tile_pool(name="const", bufs=1))
        big = ctx.enter_context(tc.tile_pool(name="big", bufs=1))
        acc = ctx.enter_context(tc.tile_pool(name="acc", bufs=1))
        dpool = ctx.enter_context(tc.tile_pool(name="dpool", bufs=6))
        npool = ctx.enter_context(tc.tile_pool(name="npool", bufs=2))
        mpsum = ctx.enter_context(tc.tile_pool(name="mpsum", bufs=3, space="PSUM"))
        tpsum = ctx.enter_context(tc.tile_pool(name="tpsum", bufs=2, space="PSUM"))
        dram = ctx.enter_context(tc.tile_pool(name="dram", bufs=1, space="DRAM"))

        id_dram = nc.inline_tensor(np.eye(128, dtype=np.float32), name="id128")
        identityd = const.tile([128, 128], f32)
        nc.sync.dma_start(out=identityd, in_=id_dram[:, :])
        identity = const.tile([128, 128], f32)
        nc.vector.tensor_copy(out=identity, in_=identityd)

        # ---- input DMAs (a in 8 chunks; first chunk leads for fast ramp) ----
        a_nat = big.tile([128, N // 128, D], f32)
        NQB = N // 128 // 8  # q-blocks per chunk
        def a_chunk(h):
            nc.sync.dma_start(
                out=a_nat[:, h * NQB:(h + 1) * NQB, :],
                in_=a_ext[h * N // 8:(h + 1) * N // 8, :]
                .rearrange("(q p) d -> p q d", p=128))
        a_chunk(0)
        b_nat = big.tile([128, NIB, D], f32)
        nc.sync.dma_start(
            out=b_nat, in_=b_ext[:, :].rearrange("(q p) d -> p q d", p=128))
        r2_vec = const.tile([128, NIB], f32)
        nc.sync.dma_start(out=r2_vec, in_=r2v_ext[:, :])
        r1r = big.tile([1, N], f32r)
        nc.sync.dma_start(out=r1r, in_=r1_ext[:, :])
        for h in range(1, 8):
            a_chunk(h)

        # ones row (K=1 lhsT for the rank-1 matmul), via ACT cast
        onesrf = const.tile([1, 128], f32)
        nc.vector.memset(onesrf, 1.0)
        ones_row = const.tile([1, 128], f32r)
        nc.scalar.copy(out=ones_row, in_=onesrf)

        # ---- b strip: m2bT = -2 * b^T (f32r via DVE cast-scale) ----
        m2bT = big.tile([128, R], f32r)
        for g in range(R // 512):
            pst = tpsum.tile([128, 512], f32, tag="tpsa")
            for k in range(4):
                q = g * 4 + k
                nc.tensor.transpose(
                    pst[:, k * 128:(k + 1) * 128], b_nat[:, q, :], identity)
            nc.vector.tensor_scalar_mul(
                m2bT[:, g * 512:(g + 1) * 512], pst, -2.0)

        # ---- a full: aT = a^T (f32r); 4 transposes per Pool copy ----
        aT = big.tile([128, N], f32r)
        for g in range(N // 512):
            pst = tpsum.tile([128, 512], f32, tag="tpsa")
            for k in range(4):
                q = g * 4 + k
                nc.tensor.transpose(
                    pst[:, k * 128:(k + 1) * 128], a_nat[:, q, :], identity)
            nc.vector.tensor_copy(
                out=aT[:, g * 512:(g + 1) * 512], in_=pst)

        # ---- accumulators ----
        dsum_all = acc.tile([128, NIB * NST], f32)      # per-tile sums of d
        colminacc = [acc.tile([128, ST], bf16, name=f"colminacc{i}")
                     for i in range(NST)]
        rowminacc = [acc.tile([128, ST], bf16, name=f"rowminacc{i}")
                     for i in range(NIB)]
        rowmin8 = const.tile([128, NIB], f32)           # per-jb row mins
        rs_in = dram.tile([NCORES, ST], f32)
        rs_out = dram.tile([1, ST], f32)

        # ---- main pass over e tiles [128, 1024] ----
        for jb in range(NIB):
            wA = m2bT[:, jb * 128:(jb + 1) * 128]
            bias = r2_vec[:, jb:jb + 1]
            for s in range(NST):
                ps = mpsum.tile([128, ST], f32, tag="mps")
                for h in range(2):
                    sl = slice(s * ST + h * 512, s * ST + (h + 1) * 512)
                    psl = ps[:, h * 512:(h + 1) * 512]
                    nc.tensor.matmul(psl, lhsT=wA, rhs=aT[:, sl],
                                     start=True, stop=False)
                    nc.tensor.matmul(psl, lhsT=ones_row, rhs=r1r[0:1, sl],
                                     start=False, stop=True)
                # sqrt -> bf16, accumulate sum(d); write directly into the
                # chain accumulator tile where possible (no init copies)
                if jb == 0:
                    tgt = colminacc[s]
                elif s == 0:
                    tgt = rowminacc[jb]
                else:
                    tgt = dpool.tile([128, ST], bf16, tag="dsc")
                t = jb * NST + s
                nc.scalar.activation(
                    out=tgt, in_=ps, func=Sqrt, bias=bias, scale=1.0,
                    accum_out=dsum_all[:, t:t + 1])
                # rowmin chain (min over strips, per jb).  For jb==0 the
                # strip tiles live in colminacc[*]; the first op (s==1)
                # seeds rowminacc[0] from strips 0 and 1.
                if s > 0:
                    src = colminacc[s] if jb == 0 else tgt
                    prev = (colminacc[0] if (jb == 0 and s == 1)
                            else rowminacc[jb])
                    nc.vector.tensor_tensor(
                        out=rowminacc[jb], in0=src, in1=prev, op=MIN)
                # colmin chain (min over jb, per strip)
                if jb > 0:
                    nc.vector.tensor_tensor(
                        out=colminacc[s], in0=tgt, in1=colminacc[s], op=MIN)
                if jb == NIB - 1:
                    # colmin for this strip is final: negate, partition-max,
                    # ship row 0 into the ReduceScatter input
                    neg = npool.tile([128, ST], bf16, tag="neg")
                    nc.vector.tensor_scalar_mul(neg, colminacc[s], -1.0)
                    par = npool.tile([128, ST], f32, tag="par")
                    nc.gpsimd.partition_all_reduce(
                        out_ap=par, in_ap=neg, channels=128,
                        reduce_op=bass_isa.ReduceOp.max)
                    nc.sync.dma_start(out=rs_in[s:s + 1, :], in_=par[0:1, :])
            nc.vector.tensor_reduce(
                out=rowmin8[:, jb:jb + 1], in_=rowminacc[jb], axis=X, op=MIN)

        # ---- local scalar partials ----
        dsum_vec = const.tile([128, 1], f32)
        nc.vector.tensor_reduce(out=dsum_vec, in_=dsum_all, axis=X, op=ADD)
        posc_vec = const.tile([128, 1], f32)
        nc.vector.tensor_reduce(out=posc_vec, in_=rowmin8, axis=X, op=ADD)
        dsum_sc = const.tile([128, 1], f32)
        nc.vector.tensor_scalar_mul(dsum_sc, dsum_vec, C0)
        combo_l = const.tile([128, 1], f32)
        nc.vector.scalar_tensor_tensor(
            out=combo_l, in0=posc_vec, scalar=C2, in1=dsum_sc,
            op0=MULT, op1=ADD)
        combo_g = const.tile([128, 1], f32)
        nc.gpsimd.partition_all_reduce(
            out_ap=combo_g, in_ap=combo_l, channels=128,
            reduce_op=bass_isa.ReduceOp.add)

        # ---- cross-core: ReduceScatter(max) of negated colmins; core c
        # receives row c (strip c reduced over cores) -- chunks are disjoint,
        # so the host-side sum of the 8 core outputs covers all of pos_r.
        nc.gpsimd.collective_compute(
            "ReduceScatter", MAX, replica_groups=groups,
            ins=[rs_in.opt()], outs=[rs_out.opt()])
        posr_neg = const.tile([1, ST], f32)
        nc.sync.dma_start(out=posr_neg, in_=rs_out)
        posr_sum = const.tile([1, 1], f32)
        nc.vector.tensor_reduce(out=posr_sum, in_=posr_neg, axis=X, op=ADD)
        fin = const.tile([1, 1], f32)
        nc.vector.scalar_tensor_tensor(
            out=fin, in0=posr_sum, scalar=C1N, in1=combo_g[0:1, :],
            op0=MULT, op1=ADD)
        nc.sync.dma_start(out=out_ext[:], in_=fin)

    if not nc.is_finalized():
        nc.finalize()
    return nc


def _get_nc():
    if "nc" not in _CACHE:
        _CACHE["nc"] = _build()
    return _CACHE["nc"]


def _in_maps(output1, output2):
    a = np.ascontiguousarray(np.asarray(output1, dtype=np.float32))
    b = np.ascontiguousarray(np.asarray(output2, dtype=np.float32))
    assert a.shape == (N, D) and b.shape == (N, D)
    r1 = (a * a).sum(axis=1, dtype=np.float32).reshape(1, N)
    maps = []
    for c in range(NCORES):
        bs = b[c * R:(c + 1) * R]
        r2 = (bs * bs).sum(axis=1, dtype=np.float32)
        r2v = np.ascontiguousarray(r2.reshape(NIB, 128).T)
        maps.append({"a": a, "b": bs, "r1": r1, "r2v": r2v})
    return maps


def _run(output1, output2, trace=False):
    from concourse.bass_utils import run_bass_kernel_spmd

    res = run_bass_kernel_spmd(
        _get_nc(), _in_maps(output1, output2), list(range(NCORES)), trace=trace)
    parts = np.array([np.asarray(res.results[c]["out"], dtype=np.float32)
                      for c in range(NCORES)])
    return np.float32(parts.sum() + CONST), res


def kernel(output1, output2):
    return _run_fast(output1, output2)


# ---------------------------------------------------------------------------
# cached fast runner (keeps the jitted sharded callable alive so repeated
# calls don't re-trace) — also used by test.py for warm timing loops.
def _get_fast_runner():
    if "runner" in _CACHE:
        return _CACHE["runner"]

    import jax
    from jax.experimental.shard_map import shard_map
    from jax.sharding import Mesh, PartitionSpec
    from concourse import bass2jax, mybir

    nc = _get_nc()
    bass2jax.install_neuronx_cc_hook()

    partition_name = (
        nc.partition_id_tensor.name if nc.partition_id_tensor else None)
    in_names, out_names, out_avals = [], [], []
    for alloc in nc.m.functions[0].allocations:
        if not isinstance(alloc, mybir.MemoryLocationSet):
            continue
        name = alloc.memorylocations[0].name
        if alloc.kind == "ExternalInput":
            if name != partition_name:
                in_names.append(name)
        elif alloc.kind == "ExternalOutput":
            out_names.append(name)
            out_avals.append(jax.core.ShapedArray(
                tuple(alloc.tensor_shape), mybir.dt.np(alloc.dtype)))
    n_params = len(in_names)
    all_in_names = list(in_names) + list(out_names)
    if partition_name is not None:
        all_in_names.append(partition_name)

    def _body(*args):
        operands = list(args)
        if partition_name is not None:
            operands.append(bass2jax.partition_id_tensor())
        return tuple(bass2jax._bass_exec_p.bind(
            *operands,
            out_avals=tuple(out_avals),
            in_names=tuple(all_in_names),
            out_names=tuple(out_names),
            lowering_input_output_aliases=(),
            sim_require_finite=True,
            sim_require_nnan=True,
            nc=nc,
        ))

    devices = jax.devices()[:NCORES]
    mesh = Mesh(np.asarray(devices), ("core",))
    n_outs = len(out_names)
    sharded = jax.jit(
        shard_map(
            _body, mesh=mesh,
            in_specs=(PartitionSpec("core"),) * (n_params + n_outs),
            out_specs=(PartitionSpec("core"),) * n_outs,
            check_rep=False,
        ),
        keep_unused=True,
    )

    in_sharding = jax.sharding.NamedSharding(mesh, PartitionSpec("core"))

    def prep(in_maps):
        concat_in = [
            np.concatenate([m[nm] for m in in_maps], axis=0)
            for nm in in_names
        ]
        concat_zeros = [
            np.zeros((NCORES * av.shape[0], *av.shape[1:]), av.dtype)
            for av in out_avals
        ]
        return [jax.device_put(x, in_sharding)
                for x in concat_in + concat_zeros]

    def call(dev_args):
        outs = sharded(*dev_args)
        jax.block_until_ready(outs)
        return outs

    def call_async(dev_args):
        return sharded(*dev_args)

    def run(in_maps):
        outs = call(prep(in_maps))
        return {
            nm: np.asarray(outs[i]).reshape(NCORES, *out_avals[i].shape)
            for i, nm in enumerate(out_names)
        }

    run.prep = prep
    run.call = call
    run.call_async = call_async
    _CACHE["runner"] = run
    return run


def _run_fast(output1, output2):
    run = _get_fast_runner()
    outs = run(_in_maps(output1, output2))
    return np.float32(np.asarray(outs["out"], dtype=np.float32).sum() + CONST)



# revision 3
# speedup vs baseline: 8.6529x; 8.6529x over previous
"""ContrastiveLoss (margin=1) on 8 trn2 NeuronCores via Bass/Tile — v2.

Math: with d = cdist(output1, output2) [N, N], pos_r = rowmin(d),
pos_c = colmin(d), every hinge term  margin - pos + d >= margin > 0,
and the excluded (argmin) entry equals exactly margin.  Hence

  loss = (1 - 1/N) + sum(d)/N^2 - (mean(pos_r) + mean(pos_c))/2

Kernel needs sum(d), rowmin(d), colmin(d) in one pass over d.

Sharding: core c owns a 1024-row strip of output2 (b) and all of
output1 (a); computes e = dist(b_strip, a_full) [1024, 8192] with the
b index on partitions.  rowmin(e) (text-side pos_c) is local; colmin
needs a partition reduce (negate + gpsimd partition_all_reduce(max))
plus a cross-core ReduceScatter(max); the final scalar partials are
summed on the host from the 8 per-core outputs.

v2 changes vs baseline:
  - [128,1024] tiles (2 PSUM banks): half the per-op fixed costs,
    half the ACT accumulator reads (187ns each).
  - r1 = rowsum(a^2), r2 = rowsum(b^2) computed on HOST, shipped as
    inputs (kills 64 DVE squares + 64 ones-matmuls + 64 ACT copies).
  - ACT writes sqrt output directly into min-chain accumulator tiles
    (kills 16 init copies).
  - colmin partition-reduce via Pool partition_all_reduce (negate +
    max) instead of 64 PE transposes + 64 DVE reduces.
  - ReduceScatter(max on negated mins) instead of AllReduce(min):
    no 1.875x collective multiplier; host sums the 8 core scalars.
    (Pool/GPSIMD cannot run generic vector ops or touch PSUM on real
    HW -- only ISA ucode ops like partition_all_reduce.)
"""

import numpy as np
from contextlib import ExitStack

N = 8192          # rows of output1 == rows of output2
D = 128           # feature dim (== max matmul contraction)
NCORES = 8
R = N // NCORES   # 1024 rows per core
NIB = R // 128    # 8 row blocks per core (jb)
ST = 1024         # a-column strip width (one [128,1024] tile = 2 PSUM banks)
NST = N // ST     # 8 strips

MARGIN = 1.0
C0 = 1.0 / (float(N) * float(N))      # scale for sum(d)
C2 = -1.0 / (2.0 * float(N))          # scale for sum(pos_c)
C1N = 1.0 / (2.0 * float(N))          # scale for NEGATED sum(pos_r)
CONST = MARGIN - MARGIN / float(N)    # 1 - 1/8192  (added on host)

# engine assignment of the min chains (tuned against TimelineSim):
POOL_COLMIN_STRIPS = ()  # walrus: generic vector ops are NOT supported on Pool
POOL_ROWMIN_JBS = ()

_CACHE = {}


def _build():
    import concourse.bass as bass
    import concourse.bacc as bacc
    import concourse.tile as tile
    from concourse import mybir
    from concourse import bass_isa

    f32 = mybir.dt.float32
    f32r = mybir.dt.float32r
    bf16 = mybir.dt.bfloat16
    X = mybir.AxisListType.X
    MIN = mybir.AluOpType.min
    MAX = mybir.AluOpType.max
    ADD = mybir.AluOpType.add
    MULT = mybir.AluOpType.mult
    Sqrt = mybir.ActivationFunctionType.Sqrt

    nc = bacc.Bacc(
        trn_type="TRN2",
        target_bir_lowering=False,
        debug=False,
        num_devices=NCORES,
    )

    a_ext = nc.dram_tensor("a", [N, D], f32, kind="ExternalInput")
    b_ext = nc.dram_tensor("b", [R, D], f32, kind="ExternalInput")
    r1_ext = nc.dram_tensor("r1", [1, N], f32r, kind="ExternalInput")
    r2v_ext = nc.dram_tensor("r2v", [128, NIB], f32, kind="ExternalInput")
    out_ext = nc.dram_tensor("out", [1, 1], f32, kind="ExternalOutput")

    groups = [list(range(NCORES))]

    with tile.TileContext(nc) as tc, ExitStack() as ctx:
        const = ctx.enter_context(tc.tile_pool(name="const", bufs=1))
        big = ctx.enter_context(tc.tile_pool(name="big", bufs=1))
        acc = ctx.enter_context(tc.tile_pool(name="acc", bufs=1))
        dpool = ctx.enter_context(tc.tile_pool(name="dpool", bufs=6))
        npool = ctx.enter_context(tc.tile_pool(name="npool", bufs=2))
        mpsum = ctx.enter_context(tc.tile_pool(name="mpsum", bufs=3, space="PSUM"))
        tpsum = ctx.enter_context(tc.tile_pool(name="tpsum", bufs=2, space="PSUM"))
        dram = ctx.enter_context(tc.tile_pool(name="dram", bufs=1, space="DRAM"))

        id_dram = nc.inline_tensor(np.eye(128, dtype=np.float32), name="id128")
        identityd = const.tile([128, 128], f32)
        nc.sync.dma_start(out=identityd, in_=id_dram[:, :])
        identity = const.tile([128, 128], f32)
        nc.vector.tensor_copy(out=identity, in_=identityd)

        # ---- input DMAs (a in 8 chunks; first chunk leads for fast ramp) ----
        a_nat = big.tile([128, N // 128, D], f32)
        NQB = N // 128 // 8  # q-blocks per chunk
        def a_chunk(h):
            nc.sync.dma_start(
                out=a_nat[:, h * NQB:(h + 1) * NQB, :],
                in_=a_ext[h * N // 8:(h + 1) * N // 8, :]
                .rearrange("(q p) d -> p q d", p=128))
        a_chunk(0)
        b_nat = big.tile([128, NIB, D], f32)
        nc.sync.dma_start(
            out=b_nat, in_=b_ext[:, :].rearrange("(q p) d -> p q d", p=128))
        r2_vec = const.tile([128, NIB], f32)
        nc.sync.dma_start(out=r2_vec, in_=r2v_ext[:, :])
        r1r = big.tile([1, N], f32r)
        nc.sync.dma_start(out=r1r, in_=r1_ext[:, :])
        for h in range(1, 8):
            a_chunk(h)

        # ones row (K=1 lhsT for the rank-1 matmul), via ACT cast
        onesrf = const.tile([1, 128], f32)
        nc.vector.memset(onesrf, 1.0)
        ones_row = const.tile([1, 128], f32r)
        nc.scalar.copy(out=ones_row, in_=onesrf)

        # ---- b strip: m2bT = -2 * b^T (f32r via DVE cast-scale) ----
        m2bT = big.tile([128, R], f32r)
        for g in range(R // 512):
            pst = tpsum.tile([128, 512], f32, tag="tpsa")
            for k in range(4):
                q = g * 4 + k
                nc.tensor.transpose(
                    pst[:, k * 128:(k + 1) * 128], b_nat[:, q, :], identity)
            nc.vector.tensor_scalar_mul(
                m2bT[:, g * 512:(g + 1) * 512], pst, -2.0)

        # ---- a full: aT = a^T (f32r); 4 transposes per Pool copy ----
        aT = big.tile([128, N], f32r)
        for g in range(N // 512):
            pst = tpsum.tile([128, 512], f32, tag="tpsa")
            for k in range(4):
                q = g * 4 + k
                nc.tensor.transpose(
                    pst[:, k * 128:(k + 1) * 128], a_nat[:, q, :], identity)
            nc.vector.tensor_copy(
                out=aT[:, g * 512:(g + 1) * 512], in_=pst)

        # ---- accumulators ----
        dsum_all = acc.tile([128, NIB * NST], f32)      # per-tile sums of d
        colminacc = [acc.tile([128, ST], bf16, name=f"colminacc{i}")
                     for i in range(NST)]
        rowminacc = [acc.tile([128, ST], bf16, name=f"rowminacc{i}")
                     for i in range(NIB)]
        rowmin8 = const.tile([128, NIB], f32)           # per-jb row mins
        rs_in = dram.tile([NCORES, ST], f32)
        rs_out = dram.tile([1, ST], f32)

        # ---- main pass over e tiles [128, 1024] ----
        for jb in range(NIB):
            wA = m2bT[:, jb * 128:(jb + 1) * 128]
            bias = r2_vec[:, jb:jb + 1]
            for s in range(NST):
                ps = mpsum.tile([128, ST], f32, tag="mps")
                for h in range(2):
                    sl = slice(s * ST + h * 512, s * ST + (h + 1) * 512)
                    psl = ps[:, h * 512:(h + 1) * 512]
                    nc.tensor.matmul(psl, lhsT=wA, rhs=aT[:, sl],
                                     start=True, stop=False)
                    nc.tensor.matmul(psl, lhsT=ones_row, rhs=r1r[0:1, sl],
                                     start=False, stop=True)
                # sqrt -> bf16, accumulate sum(d); write directly into the
                # chain accumulator tile where possible (no init copies)
                if jb == 0:
                    tgt = colminacc[s]
                elif s == 0:
                    tgt = rowminacc[jb]
                else:
                    tgt = dpool.tile([128, ST], bf16, tag="dsc")
                t = jb * NST + s
                nc.scalar.activation(
                    out=tgt, in_=ps, func=Sqrt, bias=bias, scale=1.0,
                    accum_out=dsum_all[:, t:t + 1])
                # rowmin chain (min over strips, per jb).  For jb==0 the
                # strip tiles live in colminacc[*]; the first op (s==1)
                # seeds rowminacc[0] from strips 0 and 1.
                if s > 0:
                    src = colminacc[s] if jb == 0 else tgt
                    prev = (colminacc[0] if (jb == 0 and s == 1)
                            else rowminacc[jb])
                    nc.vector.tensor_tensor(
                        out=rowminacc[jb], in0=src, in1=prev, op=MIN)
                # colmin chain (min over jb, per strip)
                if jb > 0:
                    nc.vector.tensor_tensor(
                        out=colminacc[s], in0=tgt, in1=colminacc[s], op=MIN)
                if jb == NIB - 1:
                    # colmin for this strip is final: negate, partition-max,
                    # ship row 0 into the ReduceScatter input
                    neg = npool.tile([128, ST], bf16, tag="neg")
                    nc.vector.tensor_scalar_mul(neg, colminacc[s], -1.0)
                    par = npool.tile([128, ST], f32, tag="par")
                    nc.gpsimd.partition_all_reduce(
                        out_ap=par, in_ap=neg, channels=128,
                        reduce_op=bass_isa.ReduceOp.max)
                    nc.sync.dma_start(out=rs_in[s:s + 1, :], in_=par[0:1, :])
            nc.vector.tensor_reduce(
                out=rowmin8[:, jb:jb + 1], in_=rowminacc[jb], axis=X, op=MIN)

        # ---- local scalar partials ----
        dsum_vec = const.tile([128, 1], f32)
        nc.vector.tensor_reduce(out=dsum_vec, in_=dsum_all, axis=X, op=ADD)
        posc_vec = const.tile([128, 1], f32)
        nc.vector.tensor_reduce(out=posc_vec, in_=rowmin8, axis=X, op=ADD)
        dsum_sc = const.tile([128, 1], f32)
        nc.vector.tensor_scalar_mul(dsum_sc, dsum_vec, C0)
        combo_l = const.tile([128, 1], f32)
        nc.vector.scalar_tensor_tensor(
            out=combo_l, in0=posc_vec, scalar=C2, in1=dsum_sc,
            op0=MULT, op1=ADD)
        combo_g = const.tile([128, 1], f32)
        nc.gpsimd.partition_all_reduce(
            out_ap=combo_g, in_ap=combo_l, channels=128,
            reduce_op=bass_isa.ReduceOp.add)

        # ---- cross-core: ReduceScatter(max) of negated colmins; core c
        # receives row c (strip c reduced over cores) -- chunks are disjoint,
        # so the host-side sum of the 8 core outputs covers all of pos_r.
        nc.gpsimd.collective_compute(
            "ReduceScatter", MAX, replica_groups=groups,
            ins=[rs_in.opt()], outs=[rs_out.opt()])
        posr_neg = const.tile([1, ST], f32)
        nc.sync.dma_start(out=posr_neg, in_=rs_out)
        posr_sum = const.tile([1, 1], f32)
        nc.vector.tensor_reduce(out=posr_sum, in_=posr_neg, axis=X, op=ADD)
        fin = const.tile([1, 1], f32)
        nc.vector.scalar_tensor_tensor(
            out=fin, in0=posr_sum, scalar=C1N, in1=combo_g[0:1, :],
            op0=MULT, op1=ADD)
        nc.sync.dma_start(out=out_ext[:], in_=fin)

    if not nc.is_finalized():
        nc.finalize()
    return nc


def _get_nc():
    if "nc" not in _CACHE:
        _CACHE["nc"] = _build()
    return _CACHE["nc"]


def _in_maps(output1, output2):
    a = np.ascontiguousarray(np.asarray(output1, dtype=np.float32))
    b = np.ascontiguousarray(np.asarray(output2, dtype=np.float32))
    assert a.shape == (N, D) and b.shape == (N, D)
    r1 = (a * a).sum(axis=1, dtype=np.float32).reshape(1, N)
    maps = []
    for c in range(NCORES):
        bs = b[c * R:(c + 1) * R]
        r2 = (bs * bs).sum(axis=1, dtype=np.float32)
        r2v = np.ascontiguousarray(r2.reshape(NIB, 128).T)
        maps.append({"a": a, "b": bs, "r1": r1, "r2v": r2v})
    return maps


def _run(output1, output2, trace=False):
    from concourse.bass_utils import run_bass_kernel_spmd

    res = run_bass_kernel_spmd(
        _get_nc(), _in_maps(output1, output2), list(range(NCORES)), trace=trace)
    parts = np.array([np.asarray(res.results[c]["out"], dtype=np.float32)
                      for c in range(NCORES)])
    return np.float32(parts.sum() + CONST), res


def kernel(output1, output2):
    return _run_fast(output1, output2)


# ---------------------------------------------------------------------------
# cached fast runner (keeps the jitted sharded callable alive so repeated
# calls don't re-trace) — also used by test.py for warm timing loops.
def _get_fast_runner():
    if "runner" in _CACHE:
        return _CACHE["runner"]

    import jax
    from jax.experimental.shard_map import shard_map
    from jax.sharding import Mesh, PartitionSpec
    from concourse import bass2jax, mybir

    nc = _get_nc()
    bass2jax.install_neuronx_cc_hook()

    partition_name = (
        nc.partition_id_tensor.name if nc.partition_id_tensor else None)
    in_names, in_avals, out_names, out_avals = [], [], [], []
    for alloc in nc.m.functions[0].allocations:
        if not isinstance(alloc, mybir.MemoryLocationSet):
            continue
        name = alloc.memorylocations[0].name
        if alloc.kind == "ExternalInput":
            if name != partition_name:
                in_names.append(name)
                in_avals.append(jax.core.ShapedArray(
                    tuple(alloc.tensor_shape), mybir.dt.np(alloc.dtype)))
        elif alloc.kind == "ExternalOutput":
            out_names.append(name)
            out_avals.append(jax.core.ShapedArray(
                tuple(alloc.tensor_shape), mybir.dt.np(alloc.dtype)))
    n_params = len(in_names)
    all_in_names = list(in_names) + list(out_names)
    if partition_name is not None:
        all_in_names.append(partition_name)

    def _body(*args):
        operands = list(args)
        if partition_name is not None:
            operands.append(bass2jax.partition_id_tensor())
        return tuple(bass2jax._bass_exec_p.bind(
            *operands,
            out_avals=tuple(out_avals),
            in_names=tuple(all_in_names),
            out_names=tuple(out_names),
            lowering_input_output_aliases=(),
            sim_require_finite=True,
            sim_require_nnan=True,
            nc=nc,
        ))

    devices = jax.devices()[:NCORES]
    mesh = Mesh(np.asarray(devices), ("core",))
    n_outs = len(out_names)
    f = shard_map(
        _body, mesh=mesh,
        in_specs=(PartitionSpec("core"),) * (n_params + n_outs),
        out_specs=(PartitionSpec("core"),) * n_outs,
        check_rep=False,
    )

    in_sharding = jax.sharding.NamedSharding(mesh, PartitionSpec("core"))

    # AOT-compile with the bass effect suppressed: the Compiled object takes
    # jax's C++ fast-path dispatch, which cuts per-call host overhead.
    arg_specs = [
        jax.ShapeDtypeStruct((NCORES * av.shape[0], *av.shape[1:]),
                             av.dtype, sharding=in_sharding)
        for av in in_avals + out_avals
    ]
    try:
        sharded = bass2jax.fast_dispatch_compile(
            lambda: jax.jit(f, keep_unused=True).lower(*arg_specs).compile())
    except Exception:
        sharded = jax.jit(f, keep_unused=True)

    def prep(in_maps):
        concat_in = [
            np.concatenate([m[nm] for m in in_maps], axis=0)
            for nm in in_names
        ]
        concat_zeros = [
            np.zeros((NCORES * av.shape[0], *av.shape[1:]), av.dtype)
            for av in out_avals
        ]
        return [jax.device_put(x, in_sharding)
                for x in concat_in + concat_zeros]

    def call(dev_args):
        outs = sharded(*dev_args)
        jax.block_until_ready(outs)
        return outs

    def call_async(dev_args):
        return sharded(*dev_args)

    def run(in_maps):
        outs = call(prep(in_maps))
        return {
            nm: np.asarray(outs[i]).reshape(NCORES, *out_avals[i].shape)
            for i, nm in enumerate(out_names)
        }

    run.prep = prep
    run.call = call
    run.call_async = call_async
    _CACHE["runner"] = run
    return run


def _run_fast(output1, output2):
    run = _get_fast_runner()
    outs = run(_in_maps(output1, output2))
    return np.float32(np.asarray(outs["out"], dtype=np.float32).sum() + CONST)



# revision 7
# speedup vs baseline: 9.5956x; 1.1089x over previous
"""ContrastiveLoss (margin=1) on 8 trn2 NeuronCores via Bass/Tile — v2.

Math: with d = cdist(output1, output2) [N, N], pos_r = rowmin(d),
pos_c = colmin(d), every hinge term  margin - pos + d >= margin > 0,
and the excluded (argmin) entry equals exactly margin.  Hence

  loss = (1 - 1/N) + sum(d)/N^2 - (mean(pos_r) + mean(pos_c))/2

Kernel needs sum(d), rowmin(d), colmin(d) in one pass over d.

Sharding: core c owns a 1024-row strip of output2 (b) and all of
output1 (a); computes e = dist(b_strip, a_full) [1024, 8192] with the
b index on partitions.  rowmin(e) (text-side pos_c) is local; colmin
needs a partition reduce (negate + gpsimd partition_all_reduce(max))
plus a cross-core ReduceScatter(max); the final scalar partials are
summed on the host from the 8 per-core outputs.

v2 changes vs baseline:
  - [128,1024] tiles (2 PSUM banks): half the per-op fixed costs,
    half the ACT accumulator reads (187ns each).
  - r1 = rowsum(a^2), r2 = rowsum(b^2) computed on HOST, shipped as
    inputs (kills 64 DVE squares + 64 ones-matmuls + 64 ACT copies).
  - ACT writes sqrt output directly into min-chain accumulator tiles
    (kills 16 init copies).
  - colmin partition-reduce via Pool partition_all_reduce (negate +
    max) instead of 64 PE transposes + 64 DVE reduces.
  - ReduceScatter(max on negated mins) instead of AllReduce(min):
    no 1.875x collective multiplier; host sums the 8 core scalars.
    (Pool/GPSIMD cannot run generic vector ops or touch PSUM on real
    HW -- only ISA ucode ops like partition_all_reduce.)
"""

import numpy as np
from contextlib import ExitStack

N = 8192          # rows of output1 == rows of output2
D = 128           # feature dim (== max matmul contraction)
NCORES = 4        # cores used; fewer cores -> lower per-call dispatch floor
R = N // NCORES   # rows per core
NIB = R // 128    # row blocks per core (jb)
ST = 1024         # a-column strip width (one [128,1024] tile = 2 PSUM banks)
NST = N // ST     # 8 strips
SPC = NST // NCORES  # a-column strips per core in the ReduceScatter layout

MARGIN = 1.0
C0 = 1.0 / (float(N) * float(N))      # scale for sum(d)
C2 = -1.0 / (2.0 * float(N))          # scale for sum(pos_c)
C1N = 1.0 / (2.0 * float(N))          # scale for NEGATED sum(pos_r)
CONST = MARGIN - MARGIN / float(N)    # 1 - 1/8192  (added on host)

# engine assignment of the min chains (tuned against TimelineSim):
POOL_COLMIN_STRIPS = ()  # walrus: generic vector ops are NOT supported on Pool
POOL_ROWMIN_JBS = ()

_CACHE = {}


def _build():
    import concourse.bass as bass
    import concourse.bacc as bacc
    import concourse.tile as tile
    from concourse import mybir
    from concourse import bass_isa

    f32 = mybir.dt.float32
    f32r = mybir.dt.float32r
    bf16 = mybir.dt.bfloat16
    X = mybir.AxisListType.X
    MIN = mybir.AluOpType.min
    MAX = mybir.AluOpType.max
    ADD = mybir.AluOpType.add
    MULT = mybir.AluOpType.mult
    Sqrt = mybir.ActivationFunctionType.Sqrt

    nc = bacc.Bacc(
        trn_type="TRN2",
        target_bir_lowering=False,
        debug=False,
        num_devices=NCORES,
    )

    a_ext = nc.dram_tensor("a", [N, D], f32, kind="ExternalInput")
    b_ext = nc.dram_tensor("b", [R, D], f32, kind="ExternalInput")
    r1_ext = nc.dram_tensor("r1", [1, N], f32r, kind="ExternalInput")
    r2v_ext = nc.dram_tensor("r2v", [128, NIB], f32, kind="ExternalInput")
    out_ext = nc.dram_tensor("out", [1, 1], f32, kind="ExternalOutput")

    groups = [list(range(NCORES))]

    with tile.TileContext(nc) as tc, ExitStack() as ctx:
        const = ctx.enter_context(tc.tile_pool(name="const", bufs=1))
        big = ctx.enter_context(tc.tile_pool(name="big", bufs=1))
        acc = ctx.enter_context(tc.tile_pool(name="acc", bufs=1))
        dpool = ctx.enter_context(tc.tile_pool(name="dpool", bufs=6))
        npool = ctx.enter_context(tc.tile_pool(name="npool", bufs=2))
        mpsum = ctx.enter_context(tc.tile_pool(name="mpsum", bufs=3, space="PSUM"))
        tpsum = ctx.enter_context(tc.tile_pool(name="tpsum", bufs=2, space="PSUM"))
        dram = ctx.enter_context(tc.tile_pool(name="dram", bufs=1, space="DRAM"))

        id_dram = nc.inline_tensor(np.eye(128, dtype=np.float32), name="id128")
        identityd = const.tile([128, 128], f32)
        nc.sync.dma_start(out=identityd, in_=id_dram[:, :])
        identity = const.tile([128, 128], f32)
        nc.vector.tensor_copy(out=identity, in_=identityd)

        # ---- input DMAs (a in 8 chunks; first chunk leads for fast ramp) ----
        a_nat = big.tile([128, N // 128, D], f32)
        NQB = N // 128 // 8  # q-blocks per chunk
        def a_chunk(h):
            nc.sync.dma_start(
                out=a_nat[:, h * NQB:(h + 1) * NQB, :],
                in_=a_ext[h * N // 8:(h + 1) * N // 8, :]
                .rearrange("(q p) d -> p q d", p=128))
        a_chunk(0)
        b_nat = big.tile([128, NIB, D], f32)
        nc.sync.dma_start(
            out=b_nat, in_=b_ext[:, :].rearrange("(q p) d -> p q d", p=128))
        r2_vec = const.tile([128, NIB], f32)
        nc.sync.dma_start(out=r2_vec, in_=r2v_ext[:, :])
        r1r = big.tile([1, N], f32r)
        nc.sync.dma_start(out=r1r, in_=r1_ext[:, :])
        for h in range(1, 8):
            a_chunk(h)

        # ones row (K=1 lhsT for the rank-1 matmul), via ACT cast
        onesrf = const.tile([1, 128], f32)
        nc.vector.memset(onesrf, 1.0)
        ones_row = const.tile([1, 128], f32r)
        nc.scalar.copy(out=ones_row, in_=onesrf)

        # ---- b strip: m2bT = -2 * b^T (f32r via DVE cast-scale) ----
        m2bT = big.tile([128, R], f32r)
        for g in range(R // 512):
            pst = tpsum.tile([128, 512], f32, tag="tpsa")
            for k in range(4):
                q = g * 4 + k
                nc.tensor.transpose(
                    pst[:, k * 128:(k + 1) * 128], b_nat[:, q, :], identity)
            nc.vector.tensor_scalar_mul(
                m2bT[:, g * 512:(g + 1) * 512], pst, -2.0)

        # ---- a full: aT = a^T (f32r); 4 transposes per Pool copy ----
        aT = big.tile([128, N], f32r)
        for g in range(N // 512):
            pst = tpsum.tile([128, 512], f32, tag="tpsa")
            for k in range(4):
                q = g * 4 + k
                nc.tensor.transpose(
                    pst[:, k * 128:(k + 1) * 128], a_nat[:, q, :], identity)
            nc.vector.tensor_copy(
                out=aT[:, g * 512:(g + 1) * 512], in_=pst)

        # ---- accumulators ----
        dsum_all = acc.tile([128, NIB * NST], f32)      # per-tile sums of d
        colminacc = [acc.tile([128, ST], bf16, name=f"colminacc{i}")
                     for i in range(NST)]
        rmpool = ctx.enter_context(tc.tile_pool(name="rmpool", bufs=3))
        rowminv = const.tile([128, NIB], f32)           # per-jb row mins
        rs_in = dram.tile([NCORES, SPC * ST], f32)
        rs_out = dram.tile([1, SPC * ST], f32)

        # ---- main pass over e tiles [128, 1024] ----
        for jb in range(NIB):
            wA = m2bT[:, jb * 128:(jb + 1) * 128]
            bias = r2_vec[:, jb:jb + 1]
            cur_rowmin = None
            for s in range(NST):
                ps = mpsum.tile([128, ST], f32, tag="mps")
                for h in range(2):
                    sl = slice(s * ST + h * 512, s * ST + (h + 1) * 512)
                    psl = ps[:, h * 512:(h + 1) * 512]
                    nc.tensor.matmul(psl, lhsT=wA, rhs=aT[:, sl],
                                     start=True, stop=False)
                    nc.tensor.matmul(psl, lhsT=ones_row, rhs=r1r[0:1, sl],
                                     start=False, stop=True)
                # sqrt -> bf16, accumulate sum(d); write directly into the
                # chain accumulator tile where possible (no init copies)
                if jb == 0:
                    tgt = colminacc[s]
                elif s == 0:
                    cur_rowmin = rmpool.tile([128, ST], bf16, tag="rm")
                    tgt = cur_rowmin
                else:
                    tgt = dpool.tile([128, ST], bf16, tag="dsc")
                t = jb * NST + s
                nc.scalar.activation(
                    out=tgt, in_=ps, func=Sqrt, bias=bias, scale=1.0,
                    accum_out=dsum_all[:, t:t + 1])
                # rowmin chain (min over strips, per jb).  For jb==0 the
                # strip tiles live in colminacc[*]; the first op (s==1)
                # seeds the rowmin tile from strips 0 and 1.
                if s > 0:
                    src = colminacc[s] if jb == 0 else tgt
                    if jb == 0 and s == 1:
                        cur_rowmin = rmpool.tile([128, ST], bf16, tag="rm")
                        prev = colminacc[0]
                    else:
                        prev = cur_rowmin
                    nc.vector.tensor_tensor(
                        out=cur_rowmin, in0=src, in1=prev, op=MIN)
                # colmin chain (min over jb, per strip)
                if jb > 0:
                    nc.vector.tensor_tensor(
                        out=colminacc[s], in0=tgt, in1=colminacc[s], op=MIN)
                if jb == NIB - 1:
                    # colmin for this strip is final: negate, partition-max,
                    # ship row 0 into the ReduceScatter input
                    neg = npool.tile([128, ST], bf16, tag="neg")
                    nc.vector.tensor_scalar_mul(neg, colminacc[s], -1.0)
                    par = npool.tile([128, ST], f32, tag="par")
                    nc.gpsimd.partition_all_reduce(
                        out_ap=par, in_ap=neg, channels=128,
                        reduce_op=bass_isa.ReduceOp.max)
                    nc.sync.dma_start(
                        out=rs_in[s // SPC:s // SPC + 1,
                                  (s % SPC) * ST:(s % SPC + 1) * ST],
                        in_=par[0:1, :])
            nc.vector.tensor_reduce(
                out=rowminv[:, jb:jb + 1], in_=cur_rowmin, axis=X, op=MIN)

        # ---- local scalar partials ----
        dsum_vec = const.tile([128, 1], f32)
        nc.vector.tensor_reduce(out=dsum_vec, in_=dsum_all, axis=X, op=ADD)
        posc_vec = const.tile([128, 1], f32)
        nc.vector.tensor_reduce(out=posc_vec, in_=rowminv, axis=X, op=ADD)
        dsum_sc = const.tile([128, 1], f32)
        nc.vector.tensor_scalar_mul(dsum_sc, dsum_vec, C0)
        combo_l = const.tile([128, 1], f32)
        nc.vector.scalar_tensor_tensor(
            out=combo_l, in0=posc_vec, scalar=C2, in1=dsum_sc,
            op0=MULT, op1=ADD)
        combo_g = const.tile([128, 1], f32)
        nc.gpsimd.partition_all_reduce(
            out_ap=combo_g, in_ap=combo_l, channels=128,
            reduce_op=bass_isa.ReduceOp.add)

        # ---- cross-core: ReduceScatter(max) of negated colmins; core c
        # receives row c (strip c reduced over cores) -- chunks are disjoint,
        # so the host-side sum of the 8 core outputs covers all of pos_r.
        nc.gpsimd.collective_compute(
            "ReduceScatter", MAX, replica_groups=groups,
            ins=[rs_in.opt()], outs=[rs_out.opt()])
        posr_neg = const.tile([1, SPC * ST], f32)
        nc.sync.dma_start(out=posr_neg, in_=rs_out)
        posr_sum = const.tile([1, 1], f32)
        nc.vector.tensor_reduce(out=posr_sum, in_=posr_neg, axis=X, op=ADD)
        fin = const.tile([1, 1], f32)
        nc.vector.scalar_tensor_tensor(
            out=fin, in0=posr_sum, scalar=C1N, in1=combo_g[0:1, :],
            op0=MULT, op1=ADD)
        nc.sync.dma_start(out=out_ext[:], in_=fin)

    if not nc.is_finalized():
        nc.finalize()
    return nc


def _get_nc():
    if "nc" not in _CACHE:
        _CACHE["nc"] = _build()
    return _CACHE["nc"]


def _in_maps(output1, output2):
    a = np.ascontiguousarray(np.asarray(output1, dtype=np.float32))
    b = np.ascontiguousarray(np.asarray(output2, dtype=np.float32))
    assert a.shape == (N, D) and b.shape == (N, D)
    r1 = (a * a).sum(axis=1, dtype=np.float32).reshape(1, N)
    maps = []
    for c in range(NCORES):
        bs = b[c * R:(c + 1) * R]
        r2 = (bs * bs).sum(axis=1, dtype=np.float32)
        r2v = np.ascontiguousarray(r2.reshape(NIB, 128).T)
        maps.append({"a": a, "b": bs, "r1": r1, "r2v": r2v})
    return maps


def _run(output1, output2, trace=False):
    from concourse.bass_utils import run_bass_kernel_spmd

    res = run_bass_kernel_spmd(
        _get_nc(), _in_maps(output1, output2), list(range(NCORES)), trace=trace)
    parts = np.array([np.asarray(res.results[c]["out"], dtype=np.float32)
                      for c in range(NCORES)])
    return np.float32(parts.sum() + CONST), res


def kernel(output1, output2):
    return _run_fast(output1, output2)


# ---------------------------------------------------------------------------
# cached fast runner (keeps the jitted sharded callable alive so repeated
# calls don't re-trace) — also used by test.py for warm timing loops.
def _get_fast_runner():
    if "runner" in _CACHE:
        return _CACHE["runner"]

    import jax
    from jax.experimental.shard_map import shard_map
    from jax.sharding import Mesh, PartitionSpec
    from concourse import bass2jax, mybir

    nc = _get_nc()
    bass2jax.install_neuronx_cc_hook()

    partition_name = (
        nc.partition_id_tensor.name if nc.partition_id_tensor else None)
    in_names, in_avals, out_names, out_avals = [], [], [], []
    for alloc in nc.m.functions[0].allocations:
        if not isinstance(alloc, mybir.MemoryLocationSet):
            continue
        name = alloc.memorylocations[0].name
        if alloc.kind == "ExternalInput":
            if name != partition_name:
                in_names.append(name)
                in_avals.append(jax.core.ShapedArray(
                    tuple(alloc.tensor_shape), mybir.dt.np(alloc.dtype)))
        elif alloc.kind == "ExternalOutput":
            out_names.append(name)
            out_avals.append(jax.core.ShapedArray(
                tuple(alloc.tensor_shape), mybir.dt.np(alloc.dtype)))
    n_params = len(in_names)
    all_in_names = list(in_names) + list(out_names)
    if partition_name is not None:
        all_in_names.append(partition_name)

    def _body(*args):
        operands = list(args)
        if partition_name is not None:
            operands.append(bass2jax.partition_id_tensor())
        return tuple(bass2jax._bass_exec_p.bind(
            *operands,
            out_avals=tuple(out_avals),
            in_names=tuple(all_in_names),
            out_names=tuple(out_names),
            lowering_input_output_aliases=(),
            sim_require_finite=True,
            sim_require_nnan=True,
            nc=nc,
        ))

    devices = jax.devices()[:NCORES]
    mesh = Mesh(np.asarray(devices), ("core",))
    n_outs = len(out_names)
    f = shard_map(
        _body, mesh=mesh,
        in_specs=(PartitionSpec("core"),) * (n_params + n_outs),
        out_specs=(PartitionSpec("core"),) * n_outs,
        check_rep=False,
    )

    in_sharding = jax.sharding.NamedSharding(mesh, PartitionSpec("core"))

    # AOT-compile with the bass effect suppressed: the Compiled object takes
    # jax's C++ fast-path dispatch, which cuts per-call host overhead.
    arg_specs = [
        jax.ShapeDtypeStruct((NCORES * av.shape[0], *av.shape[1:]),
                             av.dtype, sharding=in_sharding)
        for av in in_avals + out_avals
    ]
    try:
        sharded = bass2jax.fast_dispatch_compile(
            lambda: jax.jit(f, keep_unused=True).lower(*arg_specs).compile())
    except Exception:
        sharded = jax.jit(f, keep_unused=True)

    def prep(in_maps):
        concat_in = [
            np.concatenate([m[nm] for m in in_maps], axis=0)
            for nm in in_names
        ]
        concat_zeros = [
            np.zeros((NCORES * av.shape[0], *av.shape[1:]), av.dtype)
            for av in out_avals
        ]
        return [jax.device_put(x, in_sharding)
                for x in concat_in + concat_zeros]

    def call(dev_args):
        outs = sharded(*dev_args)
        jax.block_until_ready(outs)
        return outs

    def call_async(dev_args):
        return sharded(*dev_args)

    def run(in_maps):
        outs = call(prep(in_maps))
        return {
            nm: np.asarray(outs[i]).reshape(NCORES, *out_avals[i].shape)
            for i, nm in enumerate(out_names)
        }

    run.prep = prep
    run.call = call
    run.call_async = call_async
    _CACHE["runner"] = run
    return run


def _run_fast(output1, output2):
    run = _get_fast_runner()
    outs = run(_in_maps(output1, output2))
    return np.float32(np.asarray(outs["out"], dtype=np.float32).sum() + CONST)



# revision 59
# speedup vs baseline: 15.1146x; 1.5752x over previous
"""ContrastiveLoss (margin=1) on trn2 NeuronCores via Bass/Tile — v5.

Math: with d = cdist(output1, output2) [N, N], pos_r = rowmin(d),
pos_c = colmin(d), every hinge term  margin - pos + d >= margin > 0,
and the excluded (argmin) entry equals exactly margin.  Hence

  loss = (1 - 1/N) + sum(d)/N^2 - (mean(pos_r) + mean(pos_c))/2

Kernel needs sum(d), rowmin(d), colmin(d) in one pass over d.

Sharding: core c owns an R-row strip of output2 (b) and all of
output1 (a); computes e = dist(b_strip, a_full) [R, 8192] with the
b index on partitions.  rowmin(e) (text-side pos_c) is local; colmin
needs a partition reduce (negate + gpsimd partition_all_reduce(max)).
There is NO cross-core collective: each core outputs its scalar
partial plus its [1, N] negated per-column colmin, and the host does
the cross-core max + sum.  (v5 used an on-device ReduceScatter(max);
removing it cut ~150-250us/call — the collective forced per-call
cross-core sync through the tunnel, far beyond its ~21us sim cost.)

Per-call wall time through the axon tunnel is dominated by per-launch
dispatch (~0.9-1.1 ms for 8 cores, nearly kernel-independent; single
-call RTT is ~40-90 ms, amortized by pipelining).  v5 therefore:
  - AOT-compiles with bass_effect suppressed (fast_dispatch_compile)
    and calls the underlying Compiled directly -> C++ fast-path
    dispatch, no per-call Python safety-net walk.
  - Ships only a + b shards (3 operands/core incl. the output buffer);
    r1 = rowsum(a^2) is built on device from the aT transpose chunks
    (DVE square -> ones-matmul partition sum -> ACT cast copy) and
    r2 = rowsum(b^2) via ACT Square + accum_out, in partition layout.
  - Main loop runs a-column strip s OUTER, jb INNER, so each strip's
    colmin finalization (negate + Pool partition_all_reduce + rs_in
    DMA) overlaps the pass instead of serializing in the tail
    (measured ~50us/call better than jb-outer despite a near-equal
    TimelineSim estimate).
  - NCORES=8: measured best per-call throughput regime (4-core cuts
    launch count but measured slower in most windows; 2-core is
    exec-bound and clearly worse).

On-core structure (per [128, ST] tile): two 512-wide f32r matmuls per
PSUM half (dist + rank-1 r1 update), ACT sqrt -> bf16 with fused
per-tile row-sum accumulation (sum(d)), DVE bf16 min chains for
rowmin (per jb over strips) and colmin (per strip over jb), Pool
partition_all_reduce(max on negated colmin) DMA'd straight into the
packed [1, 1+N] output; host adds the closed-form constant, sums
per-core scalar partials, and max-combines the colmin vectors.
(Pool/GPSIMD cannot run generic vector ops or touch PSUM on real
HW -- only ISA ucode ops like partition_all_reduce.)
"""

import numpy as np
from contextlib import ExitStack

N = 8192          # rows of output1 == rows of output2
D = 128           # feature dim (== max matmul contraction)
NCORES = 8        # all cores: best measured per-call throughput regime
DEV0 = 0          # first jax device index to use
R = N // NCORES   # rows per core
NIB = R // 128    # row blocks per core (jb)
ST = 1024         # a-column strip width (one [128,1024] tile = 2 PSUM banks)
NST = N // ST     # 8 strips
DPB = 4           # dpool depth

MARGIN = 1.0
C0 = 1.0 / (float(N) * float(N))      # scale for sum(d)
C2 = -1.0 / (2.0 * float(N))          # scale for sum(pos_c)
C1N = 1.0 / (2.0 * float(N))          # scale for NEGATED sum(pos_r)
CONST = MARGIN - MARGIN / float(N)    # 1 - 1/8192  (added on host)

# engine assignment of the min chains (tuned against TimelineSim):
POOL_COLMIN_STRIPS = ()  # walrus: generic vector ops are NOT supported on Pool
POOL_ROWMIN_JBS = ()

_CACHE = {}


def _build():
    import concourse.bass as bass
    import concourse.bacc as bacc
    import concourse.tile as tile
    from concourse import mybir
    from concourse import bass_isa

    f32 = mybir.dt.float32
    f32r = mybir.dt.float32r
    bf16 = mybir.dt.bfloat16
    X = mybir.AxisListType.X
    MIN = mybir.AluOpType.min
    MAX = mybir.AluOpType.max
    ADD = mybir.AluOpType.add
    MULT = mybir.AluOpType.mult
    Sqrt = mybir.ActivationFunctionType.Sqrt
    Square = mybir.ActivationFunctionType.Square
    Copy = mybir.ActivationFunctionType.Copy

    nc = bacc.Bacc(
        trn_type="TRN2",
        target_bir_lowering=False,
        debug=False,
        num_devices=NCORES,
    )

    # inputs arrive HOST-PERMUTED into the on-chip [p, q, d] layout so the
    # load DMA is fully contiguous per partition (32KB rows) instead of
    # 512B strided gathers (descriptor-rate-limited at <2KB)
    a_ext = nc.dram_tensor("a", [128, (N // 128) * D], f32,
                           kind="ExternalInput")
    b_ext = nc.dram_tensor("b", [128, NIB * D], f32, kind="ExternalInput")
    # out[0] = scalar partial (sum-d + pos_c terms); out[1:] = this core's
    # negated per-column colmin over its own b rows (host does the
    # cross-core max + sum -> no collective, no cross-core sync at all)
    out_ext = nc.dram_tensor("out", [1, 1 + N], f32, kind="ExternalOutput")

    with tile.TileContext(nc) as tc, ExitStack() as ctx:
        const = ctx.enter_context(tc.tile_pool(name="const", bufs=1))
        big = ctx.enter_context(tc.tile_pool(name="big", bufs=1))
        acc = ctx.enter_context(tc.tile_pool(name="acc", bufs=1))
        npool = ctx.enter_context(tc.tile_pool(name="npool", bufs=2))
        dpool = ctx.enter_context(tc.tile_pool(name="dpool", bufs=DPB))
        rmpool = ctx.enter_context(tc.tile_pool(name="rmpool", bufs=3))
        dram = ctx.enter_context(tc.tile_pool(name="dram", bufs=1, space="DRAM"))

        id_dram = nc.inline_tensor(np.eye(128, dtype=np.float32), name="id128")
        identityd = const.tile([128, 128], f32)
        nc.sync.dma_start(out=identityd, in_=id_dram[:, :])
        identity = const.tile([128, 128], f32)
        nc.vector.tensor_copy(out=identity, in_=identityd)

        # ---- input DMAs (a in 8 chunks; first chunk leads for fast ramp) ----
        a_nat = big.tile([128, N // 128, D], f32)
        NQB = N // 128 // 8  # q-blocks per chunk
        def a_chunk(h):
            nc.sync.dma_start(
                out=a_nat[:, h * NQB:(h + 1) * NQB, :],
                in_=a_ext[:, h * NQB * D:(h + 1) * NQB * D])
        a_chunk(0)
        b_nat = big.tile([128, NIB, D], f32)
        nc.sync.dma_start(out=b_nat, in_=b_ext[:, :])
        for h in range(1, 8):
            a_chunk(h)

        # ones row (K=1 lhsT for the rank-1 matmul), via ACT cast
        onesrf = const.tile([1, 128], f32)
        nc.vector.memset(onesrf, 1.0)
        ones_row = const.tile([1, 128], f32r)
        nc.scalar.copy(out=ones_row, in_=onesrf)
        onescf = const.tile([128, 1], f32)
        nc.vector.memset(onescf, 1.0)
        ones_col = const.tile([128, 1], f32r)
        nc.scalar.copy(out=ones_col, in_=onescf)

        # ---- r2 per b-row via ACT square + accumulate (partition layout) ----
        r2_vec = const.tile([128, NIB], f32)
        sqjunk = npool.tile([128, D], f32, tag="sqj")
        for jb in range(NIB):
            nc.scalar.activation(
                out=sqjunk, in_=b_nat[:, jb, :], func=Square,
                accum_out=r2_vec[:, jb:jb + 1])

        m2bT = big.tile([128, R], f32r)
        aT = big.tile([128, N], f32r)
        r1r = big.tile([1, N], f32r)
        tpsum = ctx.enter_context(tc.tile_pool(name="tpsum", bufs=2, space="PSUM"))

        # ---- b strip: m2bT = -2 * b^T (f32r via DVE cast-scale) ----
        for g in range(R // 512):
            pst = tpsum.tile([128, 512], f32, tag="tpsa")
            for k in range(4):
                q = g * 4 + k
                nc.tensor.transpose(
                    pst[:, k * 128:(k + 1) * 128], b_nat[:, q, :], identity)
            nc.vector.tensor_scalar_mul(
                m2bT[:, g * 512:(g + 1) * 512], pst, -2.0)

        # ---- a full: aT = a^T (f32r); 4 transposes per DVE copy.
        # r1 = rowsum(a^2) on the free axis, built per 512-chunk:
        # square aT chunk (DVE), partition-sum via ones-matmul (PE),
        # cast-copy [1,512] PSUM -> r1r (ACT).
        for g in range(N // 512):
            gsl = slice(g * 512, (g + 1) * 512)
            pst = tpsum.tile([128, 512], f32, tag="tpsa")
            for k in range(4):
                q = g * 4 + k
                nc.tensor.transpose(
                    pst[:, k * 128:(k + 1) * 128], a_nat[:, q, :], identity)
            nc.vector.tensor_copy(out=aT[:, gsl], in_=pst)
            aT2c = npool.tile([128, 512], f32r, tag="sq")
            nc.vector.tensor_tensor(
                out=aT2c, in0=aT[:, gsl], in1=aT[:, gsl], op=MULT)
            psr1 = tpsum.tile([1, 512], f32, tag="r1p")
            nc.tensor.matmul(psr1, lhsT=ones_col, rhs=aT2c,
                             start=True, stop=True)
            nc.scalar.copy(out=r1r[0:1, gsl], in_=psr1)

        # ---- accumulators ----
        dsum_all = acc.tile([128, NIB * NST], f32)      # per-tile sums of d
        rowminv = const.tile([128, NIB], f32)           # per-jb row mins

        # ---- main pass over e tiles [128, ST], a-column strip s OUTER so
        # each strip's colmin finalization (negate + Pool partition-reduce +
        # output DMA) is spread across the pass instead of piling up in the
        # tail; only the cheap rowmin reduces remain at the end.
        mpsum = ctx.enter_context(tc.tile_pool(name="mpsum", bufs=2, space="PSUM"))
        rowminacc = [acc.tile([128, ST], bf16, name=f"rowminacc{i}")
                     for i in range(NIB)]
        for s in range(NST):
            cur_colmin = None
            for jb in range(NIB):
                wA = m2bT[:, jb * 128:(jb + 1) * 128]
                bias = r2_vec[:, jb:jb + 1]
                ps = mpsum.tile([128, ST], f32, tag="mps")
                for h in range(ST // 512):
                    sl = slice(s * ST + h * 512, s * ST + (h + 1) * 512)
                    psl = ps[:, h * 512:(h + 1) * 512]
                    nc.tensor.matmul(psl, lhsT=wA, rhs=aT[:, sl],
                                     start=True, stop=False)
                    nc.tensor.matmul(psl, lhsT=ones_row, rhs=r1r[0:1, sl],
                                     start=False, stop=True)
                # sqrt -> bf16, accumulate sum(d); write directly into the
                # chain accumulator tile where possible (no init copies)
                if s == 0:
                    tgt = rowminacc[jb]
                elif jb == 0:
                    cur_colmin = rmpool.tile([128, ST], bf16, tag="cm")
                    tgt = cur_colmin
                else:
                    tgt = dpool.tile([128, ST], bf16, tag="dsc")
                t = jb * NST + s
                nc.scalar.activation(
                    out=tgt, in_=ps, func=Sqrt, bias=bias, scale=1.0,
                    accum_out=dsum_all[:, t:t + 1])
                # colmin chain (min over jb, per strip).  For s==0 the jb
                # tiles live in rowminacc[*]; the first op (jb==1) seeds
                # the colmin tile from jb 0 and 1.
                if jb > 0:
                    src = rowminacc[jb] if s == 0 else tgt
                    if s == 0 and jb == 1:
                        cur_colmin = rmpool.tile([128, ST], bf16, tag="cm")
                        prev = rowminacc[0]
                    else:
                        prev = cur_colmin
                    nc.vector.tensor_tensor(
                        out=cur_colmin, in0=src, in1=prev, op=MIN)
                # rowmin chain (min over strips, per jb)
                if s > 0:
                    nc.vector.tensor_tensor(
                        out=rowminacc[jb], in0=tgt, in1=rowminacc[jb], op=MIN)
                if jb == NIB - 1:
                    # colmin for this strip is final: negate, partition-
                    # max, ship row 0 straight into the output tensor
                    neg = npool.tile([128, ST], bf16, tag="neg")
                    nc.vector.tensor_scalar_mul(neg, cur_colmin, -1.0)
                    par = npool.tile([128, ST], f32, tag="par")
                    nc.gpsimd.partition_all_reduce(
                        out_ap=par, in_ap=neg, channels=128,
                        reduce_op=bass_isa.ReduceOp.max)
                    nc.sync.dma_start(
                        out=out_ext[0:1, 1 + s * ST:1 + (s + 1) * ST],
                        in_=par[0:1, :])
                # rowmin for this jb is final once the last strip lands;
                # fold halves then reduce the narrow half inline so only
                # jb==NIB-1's work sits in the tail
                if s == NST - 1:
                    fold = npool.tile([128, ST // 2], bf16, tag="fold")
                    nc.vector.tensor_tensor(
                        out=fold, in0=rowminacc[jb][:, :ST // 2],
                        in1=rowminacc[jb][:, ST // 2:], op=MIN)
                    nc.vector.tensor_reduce(
                        out=rowminv[:, jb:jb + 1], in_=fold, axis=X, op=MIN)

        # ---- local scalar partials ----
        dsum_vec = const.tile([128, 1], f32)
        nc.vector.tensor_reduce(out=dsum_vec, in_=dsum_all, axis=X, op=ADD)
        posc_vec = const.tile([128, 1], f32)
        nc.vector.tensor_reduce(out=posc_vec, in_=rowminv, axis=X, op=ADD)
        dsum_sc = const.tile([128, 1], f32)
        nc.vector.tensor_scalar_mul(dsum_sc, dsum_vec, C0)
        combo_l = const.tile([128, 1], f32)
        nc.vector.scalar_tensor_tensor(
            out=combo_l, in0=posc_vec, scalar=C2, in1=dsum_sc,
            op0=MULT, op1=ADD)
        combo_g = const.tile([128, 1], f32)
        nc.gpsimd.partition_all_reduce(
            out_ap=combo_g, in_ap=combo_l, channels=128,
            reduce_op=bass_isa.ReduceOp.add)
        nc.sync.dma_start(out=out_ext[0:1, 0:1], in_=combo_g[0:1, :])

    if not nc.is_finalized():
        nc.finalize()
    return nc


def _get_nc():
    if "nc" not in _CACHE:
        _CACHE["nc"] = _build()
    return _CACHE["nc"]


def _perm(x):
    """[rows, D] -> [128, (rows//128)*D] in the on-chip p,q,d layout."""
    q = x.shape[0] // 128
    return np.ascontiguousarray(
        x.reshape(q, 128, D).transpose(1, 0, 2).reshape(128, q * D))


def _in_maps(output1, output2):
    a = np.asarray(output1, dtype=np.float32)
    b = np.asarray(output2, dtype=np.float32)
    assert a.shape == (N, D) and b.shape == (N, D)
    ap = _perm(a)
    return [{"a": ap, "b": _perm(b[c * R:(c + 1) * R])} for c in range(NCORES)]


def _combine(out_rows):
    """out_rows: [NCORES, 1 + N] f32 -> final loss (host-side cross-core
    max of the negated colmins replaces the on-device ReduceScatter)."""
    o = np.asarray(out_rows, dtype=np.float32).reshape(NCORES, 1 + N)
    scalars = o[:, 0]
    posr_neg = np.max(o[:, 1:], axis=0)  # -min over all b rows, per column
    return np.float32(scalars.sum(dtype=np.float64)
                      + C1N * posr_neg.sum(dtype=np.float64) + CONST)


def _run(output1, output2, trace=False):
    from concourse.bass_utils import run_bass_kernel_spmd

    res = run_bass_kernel_spmd(
        _get_nc(), _in_maps(output1, output2), list(range(NCORES)), trace=trace)
    rows = np.array([np.asarray(res.results[c]["out"], dtype=np.float32)
                     for c in range(NCORES)])
    return _combine(rows), res


def kernel(output1, output2):
    return _run_fast(output1, output2)


# ---------------------------------------------------------------------------
# cached fast runner (keeps the jitted sharded callable alive so repeated
# calls don't re-trace) — also used by test.py for warm timing loops.
def _get_fast_runner():
    if "runner" in _CACHE:
        return _CACHE["runner"]

    import jax
    from jax.experimental.shard_map import shard_map
    from jax.sharding import Mesh, PartitionSpec
    from concourse import bass2jax, mybir

    nc = _get_nc()
    bass2jax.install_neuronx_cc_hook()

    partition_name = (
        nc.partition_id_tensor.name if nc.partition_id_tensor else None)
    in_names, in_avals, out_names, out_avals = [], [], [], []
    for alloc in nc.m.functions[0].allocations:
        if not isinstance(alloc, mybir.MemoryLocationSet):
            continue
        name = alloc.memorylocations[0].name
        if alloc.kind == "ExternalInput":
            if name != partition_name:
                in_names.append(name)
                in_avals.append(jax.core.ShapedArray(
                    tuple(alloc.tensor_shape), mybir.dt.np(alloc.dtype)))
        elif alloc.kind == "ExternalOutput":
            out_names.append(name)
            out_avals.append(jax.core.ShapedArray(
                tuple(alloc.tensor_shape), mybir.dt.np(alloc.dtype)))
    n_params = len(in_names)
    all_in_names = list(in_names) + list(out_names)
    if partition_name is not None:
        all_in_names.append(partition_name)

    def _body(*args):
        operands = list(args)
        if partition_name is not None:
            operands.append(bass2jax.partition_id_tensor())
        return tuple(bass2jax._bass_exec_p.bind(
            *operands,
            out_avals=tuple(out_avals),
            in_names=tuple(all_in_names),
            out_names=tuple(out_names),
            lowering_input_output_aliases=(),
            sim_require_finite=True,
            sim_require_nnan=True,
            nc=nc,
        ))

    devices = jax.devices()[DEV0:DEV0 + NCORES]
    mesh = Mesh(np.asarray(devices), ("core",))
    n_outs = len(out_names)
    f = shard_map(
        _body, mesh=mesh,
        in_specs=(PartitionSpec("core"),) * (n_params + n_outs),
        out_specs=(PartitionSpec("core"),) * n_outs,
        check_rep=False,
    )

    in_sharding = jax.sharding.NamedSharding(mesh, PartitionSpec("core"))

    # AOT-compile with the bass effect suppressed: the Compiled object takes
    # jax's C++ fast-path dispatch, which cuts per-call host overhead.
    arg_specs = [
        jax.ShapeDtypeStruct((NCORES * av.shape[0], *av.shape[1:]),
                             av.dtype, sharding=in_sharding)
        for av in in_avals + out_avals
    ]
    try:
        sharded = bass2jax.fast_dispatch_compile(
            lambda: jax.jit(f, keep_unused=True).lower(*arg_specs).compile())
        # Hot path: call the underlying Compiled directly, skipping the
        # per-call Python safety-net shard walk (~50us/call).  Callers
        # always block_until_ready the outputs, which surfaces device
        # errors just the same.
        import jax._src.stages as jax_stages
        _base_call = jax_stages.Compiled.__call__
        _callable = lambda *args: _base_call(sharded, *args)
    except Exception:
        sharded = jax.jit(f, keep_unused=True)
        _callable = sharded

    def prep(in_maps):
        concat_in = [
            np.concatenate([m[nm] for m in in_maps], axis=0)
            for nm in in_names
        ]
        concat_zeros = [
            np.zeros((NCORES * av.shape[0], *av.shape[1:]), av.dtype)
            for av in out_avals
        ]
        return [jax.device_put(x, in_sharding)
                for x in concat_in + concat_zeros]

    def call(dev_args):
        outs = _callable(*dev_args)
        jax.block_until_ready(outs)
        return outs

    def call_async(dev_args):
        return _callable(*dev_args)

    def run(in_maps):
        outs = call(prep(in_maps))
        return {
            nm: np.asarray(outs[i]).reshape(NCORES, *out_avals[i].shape)
            for i, nm in enumerate(out_names)
        }

    run.prep = prep
    run.call = call
    run.call_async = call_async
    _CACHE["runner"] = run
    return run


def _run_fast(output1, output2):
    run = _get_fast_runner()
    outs = run(_in_maps(output1, output2))
    return _combine(outs["out"])

